# revision 7
# baseline (speedup 1.0000x reference)
"""Trainium2 Bass kernel for nn_BWCaster_86337432584570 (embedding_lookup), v3.2.

sigma[n,j] = relu( sum_p sum_c bilinear(plane_p[j])[c] * linear(line_p[j])[c] )

Design: 3 fused 256B gather rows per (point, joint) — each plane's row
carries a line's 2 exact taps (the line coordinate equals an exact row/col
index of some plane):
  row0 (y=|g1|, x=|g0|): interleaved per c: P0[c,2y,2x] (4) | L2[c, x:x+2] (2) | 2 pad
  row1 (y=|g2|, x=|g0|): per c: P1 corners | L0[c, y:y+2] | pad
  row2 (y=|g2|, x=|g1|): per c: P2 corners | L1[c, x:x+2] | pad
Row layout is c-major with 8 values per channel (4 plane + 2 line + 2 pad),
so one DVE multiply handles plane corners and line taps together.
Gathers run on 4 SWDGE queues (6 calls/iter) — the per-queue descriptor
ring feed (~7 ns/desc/queue) is the bottleneck.
"""
import sys
import numpy as np
import ml_dtypes

sys.path.insert(0, "/opt/trn_rl_repo")

import concourse.bass as bass
import concourse.bacc as bacc
import concourse.mybir as mybir
from concourse.bass_utils import run_bass_kernel_spmd
from concourse.library_config import mlp

# ---------------- problem constants (hardcoded) ----------------
N_TOTAL, J, C, G = 262144, 24, 16, 128
N_CORES = 8
NPTS = N_TOTAL // N_CORES          # 32768 points per core
SB = 4096                          # points per superblock
NSB = NPTS // SB                   # 8
NB = SB // 128                     # 32 sub-blocks of 128 points
NSLOT = 3 * NB                     # 96 gather slots per (j, superblock)
NIDX = NSLOT * 128                 # 12288 indices per (j, superblock)
DEPTH = 3                          # pipeline depth (gather buffers)
NROWS = 127 * 127                  # 16129 rows per (j, p) table
WLEN = 3 * NB * 6                  # 576 weights per partition-point
# gather sub-calls: (p, slot_lo, slot_hi, queue) — slot counts per queue equal
GCALLS = [(0, 0, 24, 0), (0, 24, 32, 1),
          (1, 0, 16, 1), (1, 16, 32, 2),
          (2, 0, 8, 2), (2, 8, 32, 3)]
BF16 = mybir.dt.bfloat16
F32 = mybir.dt.float32
I16 = mybir.dt.int16

_CACHE = {}


# ---------------- host-side prep ----------------
def _build_tables(planes, lines):
    """tab [3, J, NROWS, 16, 8] bf16; row (y,x) of table p, per channel c:
    [P[c,y:y+2,x:x+2] (4), L[c, t:t+2] (2), 0, 0]."""
    tab = np.zeros((3, J, NROWS, C, 8), dtype=np.float32)
    for p in range(3):
        sw = np.lib.stride_tricks.sliding_window_view(planes[p], (2, 2), axis=(2, 3))
        # [J, C, 127, 127, 2, 2] -> [J, y, x, C, 4]
        tab[p, :, :, :, 0:4] = sw.transpose(0, 2, 3, 1, 4, 5).reshape(J, NROWS, C, 4)
    # line tails (2 exact taps, broadcast over the unused row coordinate)
    l2 = np.lib.stride_tricks.sliding_window_view(lines[2], 2, axis=2)  # [J,C,127,2]
    t0 = np.broadcast_to(l2.transpose(0, 2, 1, 3)[:, None, :, :, :],
                         (J, 127, 127, C, 2))                  # bcast over y
    tab[0, :, :, :, 4:6] = t0.reshape(J, NROWS, C, 2)
    l0 = np.lib.stride_tricks.sliding_window_view(lines[0], 2, axis=2)
    t1 = np.broadcast_to(l0.transpose(0, 2, 1, 3)[:, :, None, :, :],
                         (J, 127, 127, C, 2))                  # bcast over x
    tab[1, :, :, :, 4:6] = t1.reshape(J, NROWS, C, 2)
    l1 = np.lib.stride_tricks.sliding_window_view(lines[1], 2, axis=2)
    t2 = np.broadcast_to(l1.transpose(0, 2, 1, 3)[:, None, :, :, :],
                         (J, 127, 127, C, 2))                  # bcast over y
    tab[2, :, :, :, 4:6] = t2.reshape(J, NROWS, C, 2)
    return tab.reshape(3, J, NROWS, 128).astype(ml_dtypes.bfloat16)


def _coords_weights(xyz, transforms):
    """Returns idx3 [N,J,3] int16 and w6 [N,J,3,6] f32
    (per row: 4 bilinear corner weights + 2 line tap weights)."""
    N = xyz.shape[0]
    xyzh = np.concatenate([xyz, np.ones((N, 1), np.float32)], axis=1)
    pts = np.einsum('jab,nb->nja', transforms[:, :3, :].astype(np.float32), xyzh)
    coord = (pts * np.float32(2.0 / 3.0) + np.float32(1.0)) * np.float32(0.5 * (G - 1))
    c0 = np.floor(coord).astype(np.int32)          # [N,J,3] per-axis cell
    fr = (coord - c0).astype(np.float32)
    c0c = np.clip(c0, 0, 126)                      # safety; margin makes this a no-op

    # (y_axis, x_axis) per plane row: row0=(1,0), row1=(2,0), row2=(2,1)
    YX = [(1, 0), (2, 0), (2, 1)]
    LAX = [0, 2, 1]   # line tap axis per row: row0=L2@g0, row1=L0@g2, row2=L1@g1
    idx3 = np.empty((N, J, 3), np.int16)
    w6 = np.empty((N, J, 3, 6), np.float32)
    for p, (ya, xa) in enumerate(YX):
        y0, x0 = c0c[:, :, ya], c0c[:, :, xa]
        fy, fx = fr[:, :, ya], fr[:, :, xa]
        idx3[:, :, p] = (y0 * 127 + x0).astype(np.int16)
        wy0, wx0 = 1.0 - fy, 1.0 - fx
        w6[:, :, p, 0] = wy0 * wx0
        w6[:, :, p, 1] = wy0 * fx
        w6[:, :, p, 2] = fy * wx0
        w6[:, :, p, 3] = fy * fx
        fl = fr[:, :, LAX[p]]
        w6[:, :, p, 4] = 1.0 - fl
        w6[:, :, p, 5] = fl
    return idx3, w6


def _pack_core(idx3, w6):
    """idx3 [NPTS,J,3] -> idx dram [J,NSB,128,NIDX//16] int16 (wrapped+replicated)
    w6 -> w dram [J,NSB,128,WLEN] bf16 layout [3p,NB nb,6]."""
    u = idx3.reshape(NSB, NB, 128, J, 3)
    arr = u.transpose(3, 0, 4, 1, 2).reshape(J, NSB, NSLOT * 128)
    wrapped = arr.reshape(J, NSB, NIDX // 16, 16).transpose(0, 1, 3, 2)
    idx_dram = np.broadcast_to(wrapped[:, :, None, :, :], (J, NSB, 8, 16, NIDX // 16))
    idx_dram = np.ascontiguousarray(idx_dram).reshape(J, NSB, 128, NIDX // 16)

    a = w6.reshape(NSB, NB, 128, J, 3, 6).transpose(3, 0, 2, 4, 1, 5)  # [j,sb,np,p,nb,6]
    w_dram = np.ascontiguousarray(a).reshape(J, NSB, 128, WLEN).astype(ml_dtypes.bfloat16)
    return idx_dram, w_dram


# ---------------- device kernel ----------------
def _build_bass(nit_lim=None):
    """nit_lim > 192 wraps (for slope timing); real workload is NIT=192."""
    nc = bacc.Bacc("TRN2", num_swdge_queues=4)
    tab = nc.dram_tensor("tab", [3, J, NROWS, 128], BF16, kind="ExternalInput")
    idx = nc.dram_tensor("idx", [J, NSB, 128, NIDX // 16], I16, kind="ExternalInput")
    w8 = nc.dram_tensor("w8", [J, NSB, 128, WLEN], BF16, kind="ExternalInput")
    # device-native layout [np, sb, nb, j]; host transposes to [NPTS, J]
    out = nc.dram_tensor("out", [128, NSB, NB, J], F32, kind="ExternalOutput")

    NITR = J * NSB  # 192 iterations, j outer / sb inner
    NIT = nit_lim if nit_lim is not None else NITR
    D = DEPTH
    NCALL = len(GCALLS)

    from contextlib import ExitStack
    with ExitStack() as ctx:
        dst = ctx.enter_context(nc.sbuf_tensor("dst", [128, D, NSLOT, 128], BF16))
        idxs = ctx.enter_context(nc.sbuf_tensor("idxs", [128, D, NIDX // 16], I16))
        w8t = ctx.enter_context(nc.sbuf_tensor("w8t", [128, D, WLEN], BF16))
        wprod = ctx.enter_context(nc.sbuf_tensor("wprod", [128, 3 * NB * 16, 6], BF16))
        t1 = ctx.enter_context(nc.sbuf_tensor("t1", [128, 3 * NB * 16, 2], BF16))
        pf = ctx.enter_context(nc.sbuf_tensor("pf", [128, 3, NB, 16], F32))
        lf = ctx.enter_context(nc.sbuf_tensor("lf", [128, 3, NB, 16], F32))
        prod = ctx.enter_context(nc.sbuf_tensor("prod", [128, NB, 3, 16], F32))
        outt = ctx.enter_context(nc.sbuf_tensor("outt", [128, NSB, NB, J], F32))
        s_gat = [ctx.enter_context(nc.semaphore(f"s_gat{i}")) for i in range(D)]
        s_idx = [ctx.enter_context(nc.semaphore(f"s_idx{i}")) for i in range(D)]
        s_w8 = [ctx.enter_context(nc.semaphore(f"s_w8{i}")) for i in range(D)]
        s_cmb = ctx.enter_context(nc.semaphore("s_cmb"))
        s_relu = ctx.enter_context(nc.semaphore("s_relu"))
        s_out = ctx.enter_context(nc.semaphore("s_out"))
        s_v = ctx.enter_context(nc.semaphore("s_v"))
        block = ctx.enter_context(nc.Block())

        @block.gpsimd
        def _(gpsimd):
            gpsimd.load_library(mlp)
            for it in range(NIT):
                e = it % NITR
                j = e // NSB
                b = it % D
                if it >= D:
                    # dst[b] free once compute(it-D) done
                    gpsimd.wait_ge(s_cmb, it - (D - 1))
                gpsimd.wait_ge(s_idx[b], 16 * (it // D + 1))
                for (p, lo, hi, q) in GCALLS:
                    s0 = p * NB + lo
                    s1 = p * NB + hi
                    n = (hi - lo) * 128
                    gpsimd.dma_gather(
                        dst[:, b, s0:s1, :], tab[p, j],
                        idxs[:, b, s0 * 8:s1 * 8],
                        n, n, 128, single_packet=False, queue_num=q,
                    ).then_inc(s_gat[b], 16)

        @block.sync
        def _(sync):
            for it in range(NIT):
                e = it % NITR
                j, sb = e // NSB, e % NSB
                b = it % D
                if it >= D:
                    # w8t[b] free once compute(it-D) done
                    sync.wait_ge(s_cmb, it - (D - 1))
                    # idxs[b] consumed once gather(it-D) completed
                    sync.wait_ge(s_gat[b], 16 * NCALL * (it // D))
                sync.dma_start(idxs[:, b, :], idx[j, sb]).then_inc(s_idx[b], 16)
                sync.dma_start(w8t[:, b, :], w8[j, sb]).then_inc(s_w8[b], 16)
            # final output DMA after relu (contiguous, same layout)
            sync.wait_ge(s_relu, 1)
            sync.dma_start(out[:], outt[:]).then_inc(s_out, 16)
            sync.wait_ge(s_out, 16)

        @block.vector
        def _(vector):
            sv = 0

            def emit(inst):
                nonlocal sv
                sv += 1
                inst.then_inc(s_v, 1)

            def barrier():
                vector.wait_ge(s_v, sv)

            emit(vector.memset(outt[:].rearrange("P a b c -> P (a b c)"), 0.0))
            barrier()
            for it in range(NIT):
                e = it % NITR
                j, sb = e // NSB, e % NSB
                b = it % D
                vector.wait_ge(s_gat[b], 16 * NCALL * (it // D + 1))
                vector.wait_ge(s_w8[b], 16 * (it // D + 1))
                # one multiply: [3p,nb | 16c,6r] * w6 [3p,nb,6] bcast c
                in0 = dst[:, b, :, :].rearrange(
                    "P (p nb) (c r) -> P (p nb) c r", p=3, c=16)[:, :, :, 0:6]
                in1 = w8t[:, b, :].rearrange(
                    "P (m r) -> P m r", r=6
                ).unsqueeze(2).broadcast_to([128, 3 * NB, 16, 6])
                wv = wprod[:].rearrange("P m r -> P (m r)").rearrange(
                    "P (m c r) -> P m c r", c=16, r=6)
                emit(vector.tensor_tensor(wv, in0, in1, mybir.AluOpType.mult))
                barrier()
                # plane tree 4 -> 2 -> 1 on r[0:4]; line 2 -> 1 on r[4:6]
                emit(vector.tensor_tensor(t1[:], wprod[:, :, 0:2], wprod[:, :, 2:4],
                                          mybir.AluOpType.add))
                lfv = lf[:].rearrange("P a b c -> P (a b c)").rearrange(
                    "P (m x) -> P m x", x=1)
                emit(vector.tensor_tensor(lfv[:, :, 0], wprod[:, :, 4],
                                          wprod[:, :, 5], mybir.AluOpType.add))
                barrier()
                pfv = pf[:].rearrange("P a b c -> P (a b c)").rearrange(
                    "P (m x) -> P m x", x=1)
                emit(vector.tensor_tensor(pfv[:, :, 0], t1[:, :, 0], t1[:, :, 1],
                                          mybir.AluOpType.add))
                barrier()
                # prod[nb, p, c] = pf[p, nb, c] * lf[rowof(p), nb, c]; rowof=[1,2,0]
                prodv = prod[:].rearrange("P nb p c -> P p nb c")
                emit(vector.tensor_tensor(
                    prodv[:, 0:2], pf[:, 0:2], lf[:, 1:3], mybir.AluOpType.mult))
                emit(vector.tensor_tensor(
                    prodv[:, 2], pf[:, 2], lf[:, 0], mybir.AluOpType.mult))
                barrier()
                vector.tensor_reduce(
                    outt[:, sb, :, j],
                    prod[:].rearrange("P nb p c -> P nb (p c)"),
                    mybir.AxisListType.X, mybir.AluOpType.add,
                ).then_inc(s_cmb, 1)
            vector.wait_ge(s_cmb, NIT)
            of = outt[:].rearrange("P a b c -> P (a b c)")
            vector.tensor_scalar_max(of, of, 0.0).then_inc(s_relu, 1)

    nc.compile()
    return nc


# ---------------- entry point ----------------
def prepare_in_maps(inputs):
    planes = [np.asarray(inputs[f"plane{i}"]) for i in range(3)]
    lines = [np.asarray(inputs[f"line{i}"]) for i in range(3)]
    tab = _build_tables(planes, lines)
    idx3, w6 = _coords_weights(
        np.asarray(inputs["xyz"]), np.asarray(inputs["transforms"]))
    in_maps = []
    for k in range(N_CORES):
        s = slice(k * NPTS, (k + 1) * NPTS)
        idx_d, w_d = _pack_core(idx3[s], w6[s])
        in_maps.append({"tab": tab, "idx": idx_d, "w8": w_d})
    return in_maps


def kernel(xyz, transforms, plane0, plane1, plane2, line0, line1, line2):
    in_maps = prepare_in_maps(dict(
        xyz=xyz, transforms=transforms, plane0=plane0, plane1=plane1,
        plane2=plane2, line0=line0, line1=line1, line2=line2))

    if "nc" not in _CACHE:
        _CACHE["nc"] = _build_bass()
    nc = _CACHE["nc"]

    _CACHE["in_maps"] = in_maps
    res = run_bass_kernel_spmd(nc, in_maps, core_ids=list(range(N_CORES)))
    outs = []
    for r in res.results:
        o = np.asarray(r["out"]).reshape(128, NSB, NB, J)
        outs.append(o.transpose(1, 2, 0, 3).reshape(NPTS, J))
    return np.concatenate(outs, axis=0).astype(np.float32)


if __name__ == "__main__":
    rng = np.random.default_rng(0)
    xyz = (rng.random((N_TOTAL, 3), np.float32) * 2 - 1).astype(np.float32)
    tr = (np.eye(4, dtype=np.float32)[None]
          + 0.05 * rng.standard_normal((J, 4, 4)).astype(np.float32))
    pl = [(0.032 * rng.standard_normal((J, C, G, G))).astype(np.float32) for _ in range(3)]
    ln = [(0.032 * rng.standard_normal((J, C, G))).astype(np.float32) for _ in range(3)]
    o = kernel(xyz, tr, pl[0], pl[1], pl[2], ln[0], ln[1], ln[2])
    print(o.shape, o.dtype, float(o.max()))
